# revision 15
# baseline (speedup 1.0000x reference)
"""Trainium2 Bass kernel for nn_Attention_35605278883932.

Shape constants (hardcoded per the problem spec):
  B=2, N=2048, C=256, H=8, P=3, PH=32, hd=32.

Sharding: 8 cores = (batch b in {0,1}) x (query quarter iq in {0..3}).
Each core receives x[b] (and the pos-softmax weights) rolled so that its
512 query rows come first; attention is permutation-invariant over the
key axis, so rolling both queries and keys only changes fp summation
order. Each core computes the full attention for its 512 queries over
all 2048 keys and all 8 heads, plus the final projection; there is no
cross-core communication.

Math reductions applied (exact, not approximations):
  - pos_attn rows are i-independent: softmax_j(ph_i - ph_j + bh) =
    softmax_j(-ph_j), so pos_attn @ v is a rank-1 per-(b,h) vector u.
  - a = (1-g) attn + g pos_attn has row sums exactly 1, so the final
    renormalization is the identity.
  - per-head (1-g_h) is folded into rows of Wo; g_h/(1-g_h) into the
    host-computed pos weights; out = attn_out @ Wo' + const row.
The tiny pos MLP (P=3) and sigmoid(gate) run on host in float64; all
O(N*C) and O(N^2) work runs on the NeuronCores.
"""

import os
import numpy as np

import concourse.bacc as bacc
import concourse.mybir as mybir
import concourse.tile as tile
from concourse.bass_utils import run_bass_kernel_spmd

B, N, C, H, P = 2, 2048, 256, 8, 3
PH = C // 8
HD = C // H              # 32
NCORES = 8
IC = N // 4              # 512 queries per core
NJT = N // 128           # 16 key tiles
F32 = mybir.dt.float32
F32R = mybir.dt.float32r

_PROGRAM_CACHE = {}


def _install_profile_shim():
    """Register the NTFF profile hook missing from this image's antenv."""
    import sys, types
    try:
        from antenv.axon_hooks import get_axon_ntff_profile_hook  # noqa: F401
        return
    except ImportError:
        pass
    try:
        import trn_agent_boot.trn_boot as tb
        hook = tb._ntff_profile_via_ctypes("/opt/axon/libaxon_pjrt.so")
    except Exception:
        hook = None
    mod = types.ModuleType("antenv.axon_hooks")
    mod.get_axon_ntff_profile_hook = lambda: hook
    mod.set_axon_ntff_profile_hook = lambda h: None
    sys.modules["antenv.axon_hooks"] = mod
    from concourse import bass_utils
    bass_utils.upload_artifacts = lambda tmpdir: tmpdir


def _build_program(debug=False):
    """Emit the per-core Bass program (identical across cores)."""
    nc = bacc.Bacc("TRN2", target_bir_lowering=False, debug=False,
                   num_devices=NCORES)
    dbg = {}
    if debug:
        dbg["xT"] = nc.dram_tensor("dbg_xT", [128, 2 * NJT * 128], F32, kind="ExternalOutput")
        dbg["qkvT0"] = nc.dram_tensor("dbg_qkvT0", [128, N], F32, kind="ExternalOutput")
        dbg["E0"] = nc.dram_tensor("dbg_E0", [128, 4 * 512], F32, kind="ExternalOutput")
        dbg["acc0"] = nc.dram_tensor("dbg_acc0", [128, 512], F32, kind="ExternalOutput")
        dbg["attT0"] = nc.dram_tensor("dbg_attT0", [128, 512], F32, kind="ExternalOutput")
        dbg["ucols"] = nc.dram_tensor("dbg_ucols", [128, 2], F32, kind="ExternalOutput")
        dbg["crow"] = nc.dram_tensor("dbg_crow", [1, C], F32, kind="ExternalOutput")
        dbg["vaug0"] = nc.dram_tensor("dbg_vaug0", [128, H * (HD + 1)], F32, kind="ExternalOutput")

    x_d = nc.dram_tensor("x", [N, C], F32, kind="ExternalInput")
    w_d = nc.dram_tensor("wbar", [N, H], F32, kind="ExternalInput")
    ws_d = nc.dram_tensor("Ws", [C, C], F32, kind="ExternalInput")
    wop_d = nc.dram_tensor("Wop", [C, C], F32, kind="ExternalInput")
    bo_d = nc.dram_tensor("bo", [1, C], F32, kind="ExternalInput")
    ones_d = nc.dram_tensor("ones", [128, 128], F32, kind="ExternalInput")
    sel4_d = nc.dram_tensor("sel4", [4, 128], F32, kind="ExternalInput")
    eye_d = nc.dram_tensor("eye", [128, 128], F32, kind="ExternalInput")
    out_d = nc.dram_tensor("out", [IC, C], F32, kind="ExternalOutput")

    EXP_SCALE = 1.0 / np.sqrt(np.float32(HD))

    with tile.TileContext(nc) as tc:
        with (
            tc.tile_pool(name="consts", bufs=1) as cpool,
            tc.tile_pool(name="data", bufs=1) as dpool,
            tc.tile_pool(name="epool", bufs=4) as epool,
            tc.tile_pool(name="ps_big", bufs=1, space="PSUM") as ps_big,
            tc.tile_pool(name="ps_av", bufs=1, space="PSUM") as ps_av,
        ):
            # ---------------- constants ----------------
            ws_sb = cpool.tile([128, 2, C], F32R, tag="ws")       # [c-chunk, cc, c']
            nc.gpsimd.dma_start(ws_sb[:], ws_d.ap().rearrange("(cc p) c -> p cc c", p=128))
            wop_sb = cpool.tile([128, 2, C], F32R, tag="wop")
            nc.gpsimd.dma_start(wop_sb[:], wop_d.ap().rearrange("(cc p) c -> p cc c", p=128))
            bo_sb = cpool.tile([1, C], F32, tag="bo")
            nc.gpsimd.dma_start(bo_sb[:], bo_d.ap())
            ones_sb = cpool.tile([128, 128], F32R, tag="ones")
            nc.gpsimd.dma_start(ones_sb[:], ones_d.ap())
            eye_sb = cpool.tile([128, 128], F32, tag="eye")
            nc.gpsimd.dma_start(eye_sb[:], eye_d.ap())
            sel4_sb = cpool.tile([4, 128], F32R, tag="sel4")
            nc.gpsimd.dma_start(sel4_sb[:], sel4_d.ap())
            w_sb = cpool.tile([128, NJT, H], F32R, tag="w")       # pos weights, j-major
            nc.gpsimd.dma_start(w_sb[:], w_d.ap().rearrange("(t p) h -> p t h", p=128))

            # ---------------- x load + transpose ----------------
            x_nat = dpool.tile([128, NJT, C], F32, tag="x_nat")   # x[t*128+p, c]
            for q in range(4):
                nc.gpsimd.dma_start(
                    x_nat[:, 4 * q:4 * (q + 1), :],
                    x_d.ap().rearrange("(t p) c -> p t c", p=128)[:, 4 * q:4 * (q + 1), :])

            # xT[p, cc, t, q] = x[t*128+q, cc*128+p]
            xT = dpool.tile([128, 2, NJT, 128], F32R, tag="xT")
            for cc in range(2):
                for t0 in range(0, NJT, 4):
                    ptr = ps_big.tile([128, 4, 512], F32, tag="s")
                    for k in range(4):
                        t = t0 + k
                        nc.tensor.transpose(
                            ptr[:, k, 0:128],
                            x_nat[:, t, cc * 128:(cc + 1) * 128],
                            eye_sb[:])
                    nc.vector.tensor_copy(
                        xT[:, cc, t0:t0 + 4, :],
                        ptr[:].rearrange("p k (f g) -> p k f g", f=4)[:, :, 0, :])

            # ---------------- qkvT (heads in partition quadrants) ----------------
            # qkvT_g[p=c' within half, i] ; head h = 4g + p//32
            qkvT = []
            for g in range(2):
                pq = ps_big.tile([128, 4, 512], F32, tag="s")
                for cc in range(2):
                    for s in range(4):
                        nc.tensor.matmul(
                            pq[:, s, :],
                            ws_sb[:, cc, g * 128:(g + 1) * 128],
                            xT[:, cc].rearrange("p t q -> p (t q)")[:, s * 512:(s + 1) * 512],
                            start=(cc == 0), stop=(cc == 1))
                qt = dpool.tile([128, N], F32R, tag=f"qkvT{g}")
                nc.vector.tensor_copy(qt[:], pq[:].rearrange("p s i -> p (s i)"))
                qkvT.append(qt)
                if debug and g == 0:
                    nc.gpsimd.dma_start(dbg["qkvT0"].ap(), qt[:])
            if debug:
                nc.gpsimd.dma_start(
                    dbg["xT"].ap(),
                    xT[:].rearrange("p a t q -> p (a t q)"))

            # ---------------- qkv natural -> v_aug ----------------
            # v_aug[p=j within tile, t, h, 0:32] = v ; [..., 32] = 1.0
            v_aug = dpool.tile([128, NJT, H, HD + 1], F32R, tag="v_aug")
            v_nat = dpool.tile([128, NJT, C], F32R, tag="v_nat")
            nc.vector.tensor_copy(
                v_aug[:, :, :, HD:HD + 1].rearrange("p t h one -> p t (h one)"),
                ones_sb[:].rearrange("p (t h) -> p t h", t=NJT))
            for t in range(NJT):
                pv = ps_big.tile([128, 4, 512], F32, tag="s")
                for cc in range(2):
                    nc.tensor.matmul(
                        pv[:, 0:2, :].rearrange("p a b -> p (a b)")[:, 0:C],
                        xT[:, cc, t, :],
                        ws_sb[:, cc, :],
                        start=(cc == 0), stop=(cc == 1))
                nc.vector.tensor_copy(
                    v_aug[:, t, :, 0:HD],
                    pv[:, 0:2, :].rearrange("p a b -> p (a b)")[:, 0:C].rearrange(
                        "p (h d) -> p h d", h=H))
                nc.vector.tensor_copy(
                    v_nat[:, t, :],
                    pv[:, 0:2, :].rearrange("p a b -> p (a b)")[:, 0:C])

            # ---------------- main attention: per head-group of 4 ----------------
            attT = []   # attn_out^T per group: [128 = (h_local, d), 512]
            for g in range(2):
                qt = qkvT[g]
                accs = [ps_av.tile([128, 512], F32, tag=f"av{r}", name=f"acc_g{g}_{r}")
                        for r in range(4)]
                for jt in range(NJT):
                    ps = ps_big.tile([128, 4, 512], F32, tag="s")
                    for r in range(4):
                        tp = (96, 0) if r == 3 else None
                        nc.tensor.matmul(
                            ps[:, r, :],
                            qt[32 * r:32 * (r + 1), jt * 128:(jt + 1) * 128],
                            qt[32 * r:32 * (r + 1), 0:IC],
                            start=True, stop=True, tile_position=tp)
                    e = epool.tile([128, 4, 512], F32R, tag="E")
                    nc.scalar.activation(
                        e[:].rearrange("p r i -> p (r i)"),
                        ps[:].rearrange("p r i -> p (r i)"),
                        mybir.ActivationFunctionType.Exp, scale=EXP_SCALE)
                    if debug and g == 0 and jt == 0:
                        nc.gpsimd.dma_start(dbg["E0"].ap(), e[:].rearrange("p r i -> p (r i)"))
                    for r in range(4):
                        nc.tensor.matmul(
                            accs[r][0:HD + 1, :],
                            v_aug[:, jt, 4 * g + r, :],
                            e[:, r, :],
                            start=(jt == 0), stop=(jt == NJT - 1))
                # epilogue for this group: divide num rows by den row
                if debug and g == 0:
                    acc_dump = dpool.tile([128, 512], F32, tag="acc_dump")
                    nc.vector.tensor_copy(acc_dump[:], accs[0][:])
                    nc.gpsimd.dma_start(dbg["acc0"].ap(), acc_dump[:])
                rden4 = dpool.tile([4, 512], F32R, tag="rden4")
                for r in range(4):
                    rden = dpool.tile([1, 512], F32, tag=f"rden{r}",
                                      name=f"rden_g{g}_{r}")
                    nc.vector.reciprocal(rden[:], accs[r][HD:HD + 1, :])
                    nc.gpsimd.dma_start(rden4[r:r + 1, :], rden[:])
                div_ps = ps_big.tile([128, 4, 512], F32, tag="s",
                                     name=f"div_ps_g{g}")
                nc.tensor.matmul(div_ps[:, 0, :], sel4_sb[:], rden4[:],
                                 start=True, stop=True)
                numsb = dpool.tile([128, 512], F32, tag="numsb")
                for r in range(4):
                    nc.vector.tensor_copy(numsb[32 * r:32 * (r + 1), :],
                                          accs[r][0:HD, :])
                at = dpool.tile([128, 512], F32R, tag=f"attT{g}")
                nc.vector.tensor_mul(at[:], numsb[:], div_ps[:, 0, :])
                attT.append(at)
                if debug and g == 0:
                    nc.gpsimd.dma_start(dbg["attT0"].ap(), at[:])

            # ---------------- pos rank-1 term: U[c', h] ----------------
            u_cols = dpool.tile([128, 2], F32R, tag="u_cols")
            for cc in range(2):
                pu = ps_av.tile([128, 512], F32, tag="av0")
                for jt in range(NJT):
                    nc.tensor.matmul(
                        pu[:, 0:H],
                        v_nat[:, jt, cc * 128:(cc + 1) * 128],
                        w_sb[:, jt, :],
                        start=(jt == 0), stop=(jt == NJT - 1))
                for r in range(4):
                    h = 4 * cc + r
                    nc.vector.tensor_copy(u_cols[32 * r:32 * (r + 1), cc:cc + 1],
                                          pu[32 * r:32 * (r + 1), h:h + 1])

            if debug:
                nc.gpsimd.dma_start(dbg["ucols"].ap(), u_cols[:])
                nc.gpsimd.dma_start(
                    dbg["vaug0"].ap(),
                    v_aug[:, 0].rearrange("p h d -> p (h d)"))
            # const row = u'' @ Wop + bo   -> [1, 256]
            pc = ps_av.tile([128, 512], F32, tag="av1")
            for cc in range(2):
                nc.tensor.matmul(pc[0:1, 0:C], u_cols[:, cc:cc + 1],
                                 wop_sb[:, cc, :], start=(cc == 0), stop=(cc == 1))
            const_row = dpool.tile([1, C], F32R, tag="const_row")
            nc.vector.tensor_add(const_row[:], pc[0:1, 0:C], bo_sb[:])
            if debug:
                nc.gpsimd.dma_start(dbg["crow"].ap(), const_row[:])

            # ---------------- final projection (natural layout) ----------------
            out_sb = dpool.tile([128, 4, C], F32, tag="out_sb")
            for ib in range(4):
                pf = ps_av.tile([128, 512], F32, tag=f"av{ib}")
                for g in range(2):
                    nc.tensor.matmul(
                        pf[:, 0:C],
                        attT[g][:, ib * 128:(ib + 1) * 128],
                        wop_sb[:, g, :],
                        start=(g == 0), stop=False)
                nc.tensor.matmul(
                    pf[:, 0:C],
                    ones_sb[0:1, :],
                    const_row[:],
                    start=False, stop=True)
                nc.vector.tensor_copy(out_sb[:, ib, :], pf[:, 0:C])

            nc.gpsimd.dma_start(
                out_d.ap().rearrange("(t p) c -> p t c", p=128), out_sb[:])

    nc.compile()
    return nc


def _host_prepare(x, pos, Ws, W1, b1, W2, b2, Wh, bh, gate, Wo, bo):
    """Host-side tiny pos-MLP + gate folding (float64)."""
    pos64 = pos.astype(np.float64)
    p = np.maximum(pos64 @ W1.astype(np.float64) + b1.astype(np.float64), 0.0)
    p = p @ W2.astype(np.float64) + b2.astype(np.float64)
    ph = p @ Wh.astype(np.float64)                      # [B, N, H]
    # pos softmax over keys j (i-independent): softmax_j(-ph[b, j, h])
    z = -ph
    z -= z.max(axis=1, keepdims=True)
    e = np.exp(z)
    wbar = e / e.sum(axis=1, keepdims=True)             # [B, N, H]
    g = 1.0 / (1.0 + np.exp(-gate.astype(np.float64)))  # [H]
    w_scaled = wbar * (g / (1.0 - g))[None, None, :]    # [B, N, H]
    row_scale = np.repeat(1.0 - g, HD)                  # [C]
    Wop = (Wo.astype(np.float64) * row_scale[:, None]).astype(np.float32)
    return w_scaled.astype(np.float32), Wop


def kernel(x, pos, Ws, W1, b1, W2, b2, Wh, bh, gate, Wo, bo):
    x = np.asarray(x, np.float32)
    pos = np.asarray(pos, np.float32)
    Ws = np.asarray(Ws, np.float32)
    W1 = np.asarray(W1, np.float32); b1 = np.asarray(b1, np.float32)
    W2 = np.asarray(W2, np.float32); b2 = np.asarray(b2, np.float32)
    Wh = np.asarray(Wh, np.float32); bh = np.asarray(bh, np.float32)
    gate = np.asarray(gate, np.float32)
    Wo = np.asarray(Wo, np.float32); bo = np.asarray(bo, np.float32)

    w_scaled, Wop = _host_prepare(x, pos, Ws, W1, b1, W2, b2, Wh, bh, gate, Wo, bo)

    profile = os.environ.get("KERNEL_PROFILE", "0") == "1"
    if profile:
        _install_profile_shim()

    debug = os.environ.get("KERNEL_DEBUG", "0") == "1"
    key = f"nc_dbg{int(debug)}"
    if key not in _PROGRAM_CACHE:
        _PROGRAM_CACHE[key] = _build_program(debug=debug)
    nc = _PROGRAM_CACHE[key]

    ones128 = np.ones((128, 128), np.float32)
    eye128 = np.eye(128, dtype=np.float32)
    sel4 = np.zeros((4, 128), np.float32)
    for r in range(4):
        sel4[r, 32 * r:32 * (r + 1)] = 1.0
    bo_2d = bo.reshape(1, C)

    in_maps = []
    for core in range(NCORES):
        b, iq = divmod(core, 4)
        shift = -IC * iq
        in_maps.append({
            "x": np.ascontiguousarray(np.roll(x[b], shift, axis=0)),
            "wbar": np.ascontiguousarray(np.roll(w_scaled[b], shift, axis=0)),
            "Ws": Ws, "Wop": Wop, "bo": bo_2d,
            "ones": ones128, "eye": eye128, "sel4": sel4,
        })

    res = run_bass_kernel_spmd(nc, in_maps, list(range(NCORES)), trace=profile)
    if profile:
        kernel.last_exec_time_ns = res.exec_time_ns
        kernel.last_mean_exec_time_ns = res.mean_exec_time_ns

    if debug:
        kernel.last_debug = res.results[0]

    out = np.empty((B, N, C), np.float32)
    for core in range(NCORES):
        b, iq = divmod(core, 4)
        out[b, IC * iq:IC * (iq + 1), :] = res.results[core]["out"]
    return out


# revision 17
# speedup vs baseline: 1.0861x; 1.0861x over previous
"""Trainium2 Bass kernel for nn_Attention_35605278883932.

Shape constants (hardcoded per the problem spec):
  B=2, N=2048, C=256, H=8, P=3, PH=32, hd=32.

Sharding: 8 cores = (batch b in {0,1}) x (query quarter iq in {0..3}).
Each core receives x[b] (and the pos-softmax weights) rolled so that its
512 query rows come first; attention is permutation-invariant over the
key axis, so rolling both queries and keys only changes fp summation
order. Each core computes the full attention for its 512 queries over
all 2048 keys and all 8 heads, plus the final projection; there is no
cross-core communication.

Math reductions applied (exact, not approximations):
  - pos_attn rows are i-independent: softmax_j(ph_i - ph_j + bh) =
    softmax_j(-ph_j), so pos_attn @ v is a rank-1 per-(b,h) vector u.
  - a = (1-g) attn + g pos_attn has row sums exactly 1, so the final
    renormalization is the identity.
  - per-head (1-g_h) is folded into rows of Wo; g_h/(1-g_h) into the
    host-computed pos weights; out = attn_out @ Wo' + const row.
The tiny pos MLP (P=3) and sigmoid(gate) run on host in float64; all
O(N*C) and O(N^2) work runs on the NeuronCores.
"""

import os
import numpy as np

import concourse.bacc as bacc
import concourse.mybir as mybir
import concourse.tile as tile
from concourse.bass_utils import run_bass_kernel_spmd

B, N, C, H, P = 2, 2048, 256, 8, 3
PH = C // 8
HD = C // H              # 32
NCORES = 8
IC = N // 4              # 512 queries per core
NJT = N // 128           # 16 key tiles
F32 = mybir.dt.float32
F32R = mybir.dt.float32r

_PROGRAM_CACHE = {}


def _install_profile_shim():
    """Register the NTFF profile hook missing from this image's antenv."""
    import sys, types
    try:
        from antenv.axon_hooks import get_axon_ntff_profile_hook  # noqa: F401
        return
    except ImportError:
        pass
    try:
        import trn_agent_boot.trn_boot as tb
        hook = tb._ntff_profile_via_ctypes("/opt/axon/libaxon_pjrt.so")
    except Exception:
        hook = None
    mod = types.ModuleType("antenv.axon_hooks")
    mod.get_axon_ntff_profile_hook = lambda: hook
    mod.set_axon_ntff_profile_hook = lambda h: None
    sys.modules["antenv.axon_hooks"] = mod
    from concourse import bass_utils
    bass_utils.upload_artifacts = lambda tmpdir: tmpdir


def _build_program(debug=False):
    """Emit the per-core Bass program (identical across cores)."""
    nc = bacc.Bacc("TRN2", target_bir_lowering=False, debug=False,
                   num_devices=NCORES)
    dbg = {}
    if debug:
        dbg["xT"] = nc.dram_tensor("dbg_xT", [128, 2 * NJT * 128], F32, kind="ExternalOutput")
        dbg["qkvT0"] = nc.dram_tensor("dbg_qkvT0", [128, N], F32, kind="ExternalOutput")
        dbg["E0"] = nc.dram_tensor("dbg_E0", [128, 4 * 512], F32, kind="ExternalOutput")
        dbg["acc0"] = nc.dram_tensor("dbg_acc0", [128, 512], F32, kind="ExternalOutput")
        dbg["attT0"] = nc.dram_tensor("dbg_attT0", [128, 512], F32, kind="ExternalOutput")
        dbg["ucols"] = nc.dram_tensor("dbg_ucols", [128, 2], F32, kind="ExternalOutput")
        dbg["crow"] = nc.dram_tensor("dbg_crow", [1, C], F32, kind="ExternalOutput")
        dbg["vaug0"] = nc.dram_tensor("dbg_vaug0", [128, H * (HD + 1)], F32, kind="ExternalOutput")

    x_d = nc.dram_tensor("x", [N, C], F32, kind="ExternalInput")
    w_d = nc.dram_tensor("wbar", [N, H], F32, kind="ExternalInput")
    ws_d = nc.dram_tensor("Ws", [C, C], F32, kind="ExternalInput")
    wop_d = nc.dram_tensor("Wop", [C, C], F32, kind="ExternalInput")
    bo_d = nc.dram_tensor("bo", [1, C], F32, kind="ExternalInput")
    ones_d = nc.dram_tensor("ones", [128, 128], F32, kind="ExternalInput")
    sel4_d = nc.dram_tensor("sel4", [4, 128], F32, kind="ExternalInput")
    eye_d = nc.dram_tensor("eye", [128, 128], F32, kind="ExternalInput")
    out_d = nc.dram_tensor("out", [IC, C], F32, kind="ExternalOutput")

    EXP_SCALE = 1.0 / np.sqrt(np.float32(HD))

    with tile.TileContext(nc) as tc:
        with (
            tc.tile_pool(name="consts", bufs=1) as cpool,
            tc.tile_pool(name="data", bufs=1) as dpool,
            tc.tile_pool(name="epool", bufs=4) as epool,
            tc.tile_pool(name="ps_big", bufs=1, space="PSUM") as ps_big,
            tc.tile_pool(name="ps_av", bufs=1, space="PSUM") as ps_av,
        ):
            # ---------------- constants ----------------
            ws_sb = cpool.tile([128, 2, C], F32R, tag="ws")       # [c-chunk, cc, c']
            nc.gpsimd.dma_start(ws_sb[:], ws_d.ap().rearrange("(cc p) c -> p cc c", p=128))
            wop_sb = cpool.tile([128, 2, C], F32R, tag="wop")
            nc.gpsimd.dma_start(wop_sb[:], wop_d.ap().rearrange("(cc p) c -> p cc c", p=128))
            bo_sb = cpool.tile([1, C], F32, tag="bo")
            nc.gpsimd.dma_start(bo_sb[:], bo_d.ap())
            ones_sb = cpool.tile([128, 128], F32R, tag="ones")
            nc.gpsimd.dma_start(ones_sb[:], ones_d.ap())
            eye_sb = cpool.tile([128, 128], F32R, tag="eye")
            nc.gpsimd.dma_start(eye_sb[:], eye_d.ap())
            sel4_sb = cpool.tile([4, 128], F32R, tag="sel4")
            nc.gpsimd.dma_start(sel4_sb[:], sel4_d.ap())
            w_sb = cpool.tile([128, NJT, H], F32R, tag="w")       # pos weights, j-major
            nc.gpsimd.dma_start(w_sb[:], w_d.ap().rearrange("(t p) h -> p t h", p=128))

            # ---------------- x load + transpose ----------------
            x_nat = dpool.tile([128, NJT, C], F32R, tag="x_nat")  # x[t*128+p, c]
            for q in range(4):
                nc.gpsimd.dma_start(
                    x_nat[:, 4 * q:4 * (q + 1), :],
                    x_d.ap().rearrange("(t p) c -> p t c", p=128)[:, 4 * q:4 * (q + 1), :])

            # xT[p, cc, t, q] = x[t*128+q, cc*128+p]
            xT = dpool.tile([128, 2, NJT, 128], F32R, tag="xT")
            for cc in range(2):
                for t0 in range(0, NJT, 4):
                    ptr = ps_big.tile([128, 4, 512], F32, tag="s")
                    for k in range(4):
                        t = t0 + k
                        nc.tensor.transpose(
                            ptr[:, k, 0:128].bitcast(F32R),
                            x_nat[:, t, cc * 128:(cc + 1) * 128],
                            eye_sb[:])
                    nc.vector.tensor_copy(
                        xT[:, cc, t0:t0 + 4, :],
                        ptr[:].rearrange("p k (f g) -> p k f g", f=4)[:, :, 0, :])

            # ---------------- qkvT (heads in partition quadrants) ----------------
            # qkvT_g[p=c' within half, i] ; head h = 4g + p//32
            qkvT = []
            for g in range(2):
                pq = ps_big.tile([128, 4, 512], F32, tag="s")
                for cc in range(2):
                    for s in range(4):
                        nc.tensor.matmul(
                            pq[:, s, :],
                            ws_sb[:, cc, g * 128:(g + 1) * 128],
                            xT[:, cc].rearrange("p t q -> p (t q)")[:, s * 512:(s + 1) * 512],
                            start=(cc == 0), stop=(cc == 1))
                qt = dpool.tile([128, N], F32R, tag=f"qkvT{g}")
                nc.vector.tensor_copy(qt[:], pq[:].rearrange("p s i -> p (s i)"))
                qkvT.append(qt)
                if debug and g == 0:
                    nc.gpsimd.dma_start(dbg["qkvT0"].ap(), qt[:])
            if debug:
                nc.gpsimd.dma_start(
                    dbg["xT"].ap(),
                    xT[:].rearrange("p a t q -> p (a t q)"))

            # ---------------- qkv natural -> v_aug ----------------
            # v_aug[p=j within tile, t, h, 0:32] = v ; [..., 32] = 1.0
            v_aug = dpool.tile([128, NJT, H, HD + 1], F32R, tag="v_aug")
            v_nat = dpool.tile([128, NJT, C], F32R, tag="v_nat")
            nc.vector.tensor_copy(
                v_aug[:, :, :, HD:HD + 1].rearrange("p t h one -> p t (h one)"),
                ones_sb[:].rearrange("p (t h) -> p t h", t=NJT))
            for t in range(NJT):
                pv = ps_av.tile([128, 512], F32, tag=f"av{t % 4}", name=f"pv_{t}")
                for cc in range(2):
                    nc.tensor.matmul(
                        pv[:, 0:C],
                        xT[:, cc, t, :],
                        ws_sb[:, cc, :],
                        start=(cc == 0), stop=(cc == 1))
                nc.vector.tensor_copy(
                    v_aug[:, t, :, 0:HD],
                    pv[:, 0:C].rearrange("p (h d) -> p h d", h=H))
                nc.vector.tensor_copy(v_nat[:, t, :], pv[:, 0:C])

            # ---------------- main attention: per head-group of 4 ----------------
            attT = []   # attn_out^T per group: [128 = (h_local, d), 512]
            for g in range(2):
                qt = qkvT[g]
                accs = [ps_av.tile([128, 512], F32, tag=f"av{r}", name=f"acc_g{g}_{r}")
                        for r in range(4)]
                for jt in range(NJT):
                    ps = ps_big.tile([128, 4, 512], F32, tag="s")
                    e = epool.tile([128, 4, 512], F32R, tag="E")
                    for half in range(2):
                        for r in (2 * half, 2 * half + 1):
                            tp = (96, 0) if r == 3 else None
                            nc.tensor.matmul(
                                ps[:, r, :],
                                qt[32 * r:32 * (r + 1), jt * 128:(jt + 1) * 128],
                                qt[32 * r:32 * (r + 1), 0:IC],
                                start=True, stop=True, tile_position=tp)
                        nc.scalar.activation(
                            e[:, 2 * half:2 * half + 2, :].rearrange("p r i -> p (r i)"),
                            ps[:, 2 * half:2 * half + 2, :].rearrange("p r i -> p (r i)"),
                            mybir.ActivationFunctionType.Exp, scale=EXP_SCALE)
                    if debug and g == 0 and jt == 0:
                        nc.gpsimd.dma_start(dbg["E0"].ap(), e[:].rearrange("p r i -> p (r i)"))
                    for r in range(4):
                        nc.tensor.matmul(
                            accs[r][0:HD + 1, :],
                            v_aug[:, jt, 4 * g + r, :],
                            e[:, r, :],
                            start=(jt == 0), stop=(jt == NJT - 1))
                # epilogue for this group: divide num rows by den row
                if debug and g == 0:
                    acc_dump = dpool.tile([128, 512], F32, tag="acc_dump")
                    nc.vector.tensor_copy(acc_dump[:], accs[0][:])
                    nc.gpsimd.dma_start(dbg["acc0"].ap(), acc_dump[:])
                rden4 = dpool.tile([4, 512], F32R, tag="rden4")
                for r in range(4):
                    rden = dpool.tile([1, 512], F32, tag=f"rden{r}",
                                      name=f"rden_g{g}_{r}")
                    nc.vector.reciprocal(rden[:], accs[r][HD:HD + 1, :])
                    nc.gpsimd.dma_start(rden4[r:r + 1, :], rden[:])
                div_ps = ps_big.tile([128, 4, 512], F32, tag="s",
                                     name=f"div_ps_g{g}")
                nc.tensor.matmul(div_ps[:, 0, :], sel4_sb[:], rden4[:],
                                 start=True, stop=True)
                numsb = dpool.tile([128, 512], F32, tag="numsb")
                for r in range(4):
                    nc.vector.tensor_copy(numsb[32 * r:32 * (r + 1), :],
                                          accs[r][0:HD, :])
                at = dpool.tile([128, 512], F32R, tag=f"attT{g}")
                nc.vector.tensor_mul(at[:], numsb[:], div_ps[:, 0, :])
                attT.append(at)
                if debug and g == 0:
                    nc.gpsimd.dma_start(dbg["attT0"].ap(), at[:])

            # ---------------- pos rank-1 term: U[c', h] ----------------
            u_cols = dpool.tile([128, 2], F32R, tag="u_cols")
            for cc in range(2):
                pu = ps_av.tile([128, 512], F32, tag=f"av{cc}", name=f"pu_{cc}")
                for jt in range(NJT):
                    nc.tensor.matmul(
                        pu[:, 0:H],
                        v_nat[:, jt, cc * 128:(cc + 1) * 128],
                        w_sb[:, jt, :],
                        start=(jt == 0), stop=(jt == NJT - 1))
                for r in range(4):
                    h = 4 * cc + r
                    nc.vector.tensor_copy(u_cols[32 * r:32 * (r + 1), cc:cc + 1],
                                          pu[32 * r:32 * (r + 1), h:h + 1])

            if debug:
                nc.gpsimd.dma_start(dbg["ucols"].ap(), u_cols[:])
                nc.gpsimd.dma_start(
                    dbg["vaug0"].ap(),
                    v_aug[:, 0].rearrange("p h d -> p (h d)"))
            # const row = u'' @ Wop + bo   -> [1, 256]
            pc = ps_av.tile([128, 512], F32, tag="av2", name="pc")
            for cc in range(2):
                nc.tensor.matmul(pc[0:1, 0:C], u_cols[:, cc:cc + 1],
                                 wop_sb[:, cc, :], start=(cc == 0), stop=(cc == 1))
            const_row = dpool.tile([1, C], F32R, tag="const_row")
            nc.vector.tensor_add(const_row[:], pc[0:1, 0:C], bo_sb[:])
            if debug:
                nc.gpsimd.dma_start(dbg["crow"].ap(), const_row[:])

            # ---------------- final projection (natural layout) ----------------
            out_sb = dpool.tile([128, 4, C], F32, tag="out_sb")
            for ib in range(4):
                pf = ps_av.tile([128, 512], F32, tag=f"av{ib}")
                for g in range(2):
                    nc.tensor.matmul(
                        pf[:, 0:C],
                        attT[g][:, ib * 128:(ib + 1) * 128],
                        wop_sb[:, g, :],
                        start=(g == 0), stop=False)
                nc.tensor.matmul(
                    pf[:, 0:C],
                    ones_sb[0:1, :],
                    const_row[:],
                    start=False, stop=True)
                nc.vector.tensor_copy(out_sb[:, ib, :], pf[:, 0:C])

            nc.gpsimd.dma_start(
                out_d.ap().rearrange("(t p) c -> p t c", p=128), out_sb[:])

    nc.compile()
    return nc


def _host_prepare(x, pos, Ws, W1, b1, W2, b2, Wh, bh, gate, Wo, bo):
    """Host-side tiny pos-MLP + gate folding (float64)."""
    pos64 = pos.astype(np.float64)
    p = np.maximum(pos64 @ W1.astype(np.float64) + b1.astype(np.float64), 0.0)
    p = p @ W2.astype(np.float64) + b2.astype(np.float64)
    ph = p @ Wh.astype(np.float64)                      # [B, N, H]
    # pos softmax over keys j (i-independent): softmax_j(-ph[b, j, h])
    z = -ph
    z -= z.max(axis=1, keepdims=True)
    e = np.exp(z)
    wbar = e / e.sum(axis=1, keepdims=True)             # [B, N, H]
    g = 1.0 / (1.0 + np.exp(-gate.astype(np.float64)))  # [H]
    w_scaled = wbar * (g / (1.0 - g))[None, None, :]    # [B, N, H]
    row_scale = np.repeat(1.0 - g, HD)                  # [C]
    Wop = (Wo.astype(np.float64) * row_scale[:, None]).astype(np.float32)
    return w_scaled.astype(np.float32), Wop


def kernel(x, pos, Ws, W1, b1, W2, b2, Wh, bh, gate, Wo, bo):
    x = np.asarray(x, np.float32)
    pos = np.asarray(pos, np.float32)
    Ws = np.asarray(Ws, np.float32)
    W1 = np.asarray(W1, np.float32); b1 = np.asarray(b1, np.float32)
    W2 = np.asarray(W2, np.float32); b2 = np.asarray(b2, np.float32)
    Wh = np.asarray(Wh, np.float32); bh = np.asarray(bh, np.float32)
    gate = np.asarray(gate, np.float32)
    Wo = np.asarray(Wo, np.float32); bo = np.asarray(bo, np.float32)

    w_scaled, Wop = _host_prepare(x, pos, Ws, W1, b1, W2, b2, Wh, bh, gate, Wo, bo)

    profile = os.environ.get("KERNEL_PROFILE", "0") == "1"
    if profile:
        _install_profile_shim()

    debug = os.environ.get("KERNEL_DEBUG", "0") == "1"
    key = f"nc_dbg{int(debug)}"
    if key not in _PROGRAM_CACHE:
        _PROGRAM_CACHE[key] = _build_program(debug=debug)
    nc = _PROGRAM_CACHE[key]

    ones128 = np.ones((128, 128), np.float32)
    eye128 = np.eye(128, dtype=np.float32)
    sel4 = np.zeros((4, 128), np.float32)
    for r in range(4):
        sel4[r, 32 * r:32 * (r + 1)] = 1.0
    bo_2d = bo.reshape(1, C)

    in_maps = []
    for core in range(NCORES):
        b, iq = divmod(core, 4)
        shift = -IC * iq
        in_maps.append({
            "x": np.ascontiguousarray(np.roll(x[b], shift, axis=0)),
            "wbar": np.ascontiguousarray(np.roll(w_scaled[b], shift, axis=0)),
            "Ws": Ws, "Wop": Wop, "bo": bo_2d,
            "ones": ones128, "eye": eye128, "sel4": sel4,
        })

    res = run_bass_kernel_spmd(nc, in_maps, list(range(NCORES)), trace=profile)
    if profile:
        kernel.last_exec_time_ns = res.exec_time_ns
        kernel.last_mean_exec_time_ns = res.mean_exec_time_ns

    if debug:
        kernel.last_debug = res.results[0]

    out = np.empty((B, N, C), np.float32)
    for core in range(NCORES):
        b, iq = divmod(core, 4)
        out[b, IC * iq:IC * (iq + 1), :] = res.results[core]["out"]
    return out


# revision 20
# speedup vs baseline: 1.1648x; 1.0725x over previous
"""Trainium2 Bass kernel for nn_Attention_35605278883932.

Shape constants (hardcoded per the problem spec):
  B=2, N=2048, C=256, H=8, P=3, PH=32, hd=32.

Sharding: 8 cores = (batch b in {0,1}) x (query quarter iq in {0..3}).
Each core receives x[b] (and the pos-softmax weights) rolled so that its
512 query rows come first; attention is permutation-invariant over the
key axis, so rolling both queries and keys only changes fp summation
order. Each core computes the full attention for its 512 queries over
all 2048 keys and all 8 heads, plus the final projection; there is no
cross-core communication.

Math reductions applied (exact, not approximations):
  - pos_attn rows are i-independent: softmax_j(ph_i - ph_j + bh) =
    softmax_j(-ph_j), so pos_attn @ v is a rank-1 per-(b,h) vector u.
  - a = (1-g) attn + g pos_attn has row sums exactly 1, so the final
    renormalization is the identity.
  - per-head (1-g_h) is folded into rows of Wo; g_h/(1-g_h) into the
    host-computed pos weights; out = attn_out @ Wo' + const row.
The tiny pos MLP (P=3) and sigmoid(gate) run on host in float64; all
O(N*C) and O(N^2) work runs on the NeuronCores.
"""

import os
import numpy as np

import concourse.bacc as bacc
import concourse.mybir as mybir
import concourse.tile as tile
from concourse.bass_utils import run_bass_kernel_spmd

B, N, C, H, P = 2, 2048, 256, 8, 3
PH = C // 8
HD = C // H              # 32
NCORES = 8
IC = N // 4              # 512 queries per core
NJT = N // 128           # 16 key tiles
F32 = mybir.dt.float32
F32R = mybir.dt.float32r

_PROGRAM_CACHE = {}


def _install_profile_shim():
    """Register the NTFF profile hook missing from this image's antenv."""
    import sys, types
    try:
        from antenv.axon_hooks import get_axon_ntff_profile_hook  # noqa: F401
        return
    except ImportError:
        pass
    try:
        import trn_agent_boot.trn_boot as tb
        hook = tb._ntff_profile_via_ctypes("/opt/axon/libaxon_pjrt.so")
    except Exception:
        hook = None
    mod = types.ModuleType("antenv.axon_hooks")
    mod.get_axon_ntff_profile_hook = lambda: hook
    mod.set_axon_ntff_profile_hook = lambda h: None
    sys.modules["antenv.axon_hooks"] = mod
    from concourse import bass_utils
    bass_utils.upload_artifacts = lambda tmpdir: tmpdir


def _build_program(debug=False):
    """Emit the per-core Bass program (identical across cores)."""
    nc = bacc.Bacc("TRN2", target_bir_lowering=False, debug=False,
                   num_devices=NCORES)
    dbg = {}
    if debug:
        dbg["xT"] = nc.dram_tensor("dbg_xT", [128, 2 * NJT * 128], F32, kind="ExternalOutput")
        dbg["qkvT0"] = nc.dram_tensor("dbg_qkvT0", [128, N], F32, kind="ExternalOutput")
        dbg["E0"] = nc.dram_tensor("dbg_E0", [128, 4 * 512], F32, kind="ExternalOutput")
        dbg["acc0"] = nc.dram_tensor("dbg_acc0", [128, 512], F32, kind="ExternalOutput")
        dbg["attT0"] = nc.dram_tensor("dbg_attT0", [128, 512], F32, kind="ExternalOutput")
        dbg["ucols"] = nc.dram_tensor("dbg_ucols", [128, 2], F32, kind="ExternalOutput")
        dbg["crow"] = nc.dram_tensor("dbg_crow", [1, C], F32, kind="ExternalOutput")
        dbg["vaug0"] = nc.dram_tensor("dbg_vaug0", [128, H * (HD + 1)], F32, kind="ExternalOutput")

    x_d = nc.dram_tensor("x", [N, C], F32, kind="ExternalInput")
    w_d = nc.dram_tensor("wbar", [N, H], F32, kind="ExternalInput")
    ws_d = nc.dram_tensor("Ws", [C, C], F32, kind="ExternalInput")
    wop_d = nc.dram_tensor("Wop", [C, C], F32, kind="ExternalInput")
    bo_d = nc.dram_tensor("bo", [1, C], F32, kind="ExternalInput")
    ones_d = nc.dram_tensor("ones", [128, 128], F32, kind="ExternalInput")
    sel4_d = nc.dram_tensor("sel4", [4, 128], F32, kind="ExternalInput")
    eye_d = nc.dram_tensor("eye", [128, 128], F32, kind="ExternalInput")
    out_d = nc.dram_tensor("out", [IC, C], F32, kind="ExternalOutput")

    EXP_SCALE = 1.0 / np.sqrt(np.float32(HD))

    with tile.TileContext(nc) as tc:
        with (
            tc.tile_pool(name="consts", bufs=1) as cpool,
            tc.tile_pool(name="data", bufs=1) as dpool,
            tc.tile_pool(name="epool", bufs=4) as epool,
            tc.tile_pool(name="ps_big", bufs=1, space="PSUM") as ps_big,
            tc.tile_pool(name="ps_av", bufs=1, space="PSUM") as ps_av,
        ):
            # ---------------- constants ----------------
            ws_sb = cpool.tile([128, 2, C], F32R, tag="ws")       # [c-chunk, cc, c']
            nc.gpsimd.dma_start(ws_sb[:], ws_d.ap().rearrange("(cc p) c -> p cc c", p=128))
            wop_sb = cpool.tile([128, 2, C], F32R, tag="wop")
            nc.gpsimd.dma_start(wop_sb[:], wop_d.ap().rearrange("(cc p) c -> p cc c", p=128))
            bo_sb = cpool.tile([1, C], F32, tag="bo")
            nc.gpsimd.dma_start(bo_sb[:], bo_d.ap())
            ones_sb = cpool.tile([128, 128], F32R, tag="ones")
            nc.gpsimd.dma_start(ones_sb[:], ones_d.ap())
            eye_sb = cpool.tile([128, 128], F32R, tag="eye")
            nc.gpsimd.dma_start(eye_sb[:], eye_d.ap())
            sel4_sb = cpool.tile([4, 128], F32R, tag="sel4")
            nc.gpsimd.dma_start(sel4_sb[:], sel4_d.ap())
            w_sb = cpool.tile([128, NJT, H], F32R, tag="w")       # pos weights, j-major
            nc.gpsimd.dma_start(w_sb[:], w_d.ap().rearrange("(t p) h -> p t h", p=128))

            # ---------------- x load + transpose ----------------
            x_nat = dpool.tile([128, NJT, C], F32R, tag="x_nat")  # x[t*128+p, c]
            for q in range(4):
                nc.gpsimd.dma_start(
                    x_nat[:, 4 * q:4 * (q + 1), :],
                    x_d.ap().rearrange("(t p) c -> p t c", p=128)[:, 4 * q:4 * (q + 1), :])

            # xT[p, cc, t, q] = x[t*128+q, cc*128+p]
            xT = dpool.tile([128, 2, NJT, 128], F32R, tag="xT")
            for cc in range(2):
                for t0 in range(0, NJT, 4):
                    ptr = ps_big.tile([128, 4, 512], F32, tag="s")
                    for k in range(4):
                        t = t0 + k
                        nc.tensor.transpose(
                            ptr[:, k, 0:128].bitcast(F32R),
                            x_nat[:, t, cc * 128:(cc + 1) * 128],
                            eye_sb[:])
                    nc.scalar.copy(
                        xT[:, cc, t0:t0 + 4, :],
                        ptr[:].rearrange("p k (f g) -> p k f g", f=4)[:, :, 0, :])

            # ---------------- qkvT (heads in partition quadrants) ----------------
            # qkvT_g[p=c' within half, i] ; head h = 4g + p//32
            qkvT = []
            for g in range(2):
                pq = ps_big.tile([128, 4, 512], F32, tag="s")
                for cc in range(2):
                    for s in range(4):
                        nc.tensor.matmul(
                            pq[:, s, :],
                            ws_sb[:, cc, g * 128:(g + 1) * 128],
                            xT[:, cc].rearrange("p t q -> p (t q)")[:, s * 512:(s + 1) * 512],
                            start=(cc == 0), stop=(cc == 1))
                qt = dpool.tile([128, N], F32R, tag=f"qkvT{g}")
                nc.vector.tensor_copy(qt[:, 0:1024],
                                      pq[:].rearrange("p s i -> p (s i)")[:, 0:1024])
                nc.scalar.copy(qt[:, 1024:2048],
                               pq[:].rearrange("p s i -> p (s i)")[:, 1024:2048])
                qkvT.append(qt)
                if debug and g == 0:
                    nc.gpsimd.dma_start(dbg["qkvT0"].ap(), qt[:])
            if debug:
                nc.gpsimd.dma_start(
                    dbg["xT"].ap(),
                    xT[:].rearrange("p a t q -> p (a t q)"))

            # ---------------- qkv natural -> v_aug ----------------
            # v_aug[p=j within tile, t, h, 0:32] = v ; [..., 32] = 1.0
            v_aug = dpool.tile([128, NJT, H, HD + 1], F32R, tag="v_aug")
            v_nat = dpool.tile([128, NJT, C], F32R, tag="v_nat")
            nc.vector.tensor_copy(
                v_aug[:, :, :, HD:HD + 1].rearrange("p t h one -> p t (h one)"),
                ones_sb[:].rearrange("p (t h) -> p t h", t=NJT))
            for t in range(NJT):
                pv = ps_av.tile([128, 512], F32, tag=f"av{t % 4}", name=f"pv_{t}")
                for cc in range(2):
                    nc.tensor.matmul(
                        pv[:, 0:C],
                        xT[:, cc, t, :],
                        ws_sb[:, cc, :],
                        start=(cc == 0), stop=(cc == 1))
                nc.vector.tensor_copy(
                    v_aug[:, t, :, 0:HD],
                    pv[:, 0:C].rearrange("p (h d) -> p h d", h=H))
                nc.scalar.copy(v_nat[:, t, :], pv[:, 0:C])

            # ---------------- main attention: per head-group of 4 ----------------
            attT = []   # attn_out^T per group: [128 = (h_local, d), 512]
            for g in range(2):
                qt = qkvT[g]
                accs = [ps_av.tile([128, 512], F32, tag=f"av{r}", name=f"acc_g{g}_{r}")
                        for r in range(4)]
                for jt in range(NJT):
                    ps = ps_big.tile([128, 4, 512], F32, tag="s")
                    e = epool.tile([128, 4, 512], F32R, tag="E")
                    for half in range(2):
                        for r in (2 * half, 2 * half + 1):
                            tp = (96, 0) if r == 3 else None
                            nc.tensor.matmul(
                                ps[:, r, :],
                                qt[32 * r:32 * (r + 1), jt * 128:(jt + 1) * 128],
                                qt[32 * r:32 * (r + 1), 0:IC],
                                start=True, stop=True, tile_position=tp)
                        nc.scalar.activation(
                            e[:, 2 * half:2 * half + 2, :].rearrange("p r i -> p (r i)"),
                            ps[:, 2 * half:2 * half + 2, :].rearrange("p r i -> p (r i)"),
                            mybir.ActivationFunctionType.Exp, scale=EXP_SCALE)
                    if debug and g == 0 and jt == 0:
                        nc.gpsimd.dma_start(dbg["E0"].ap(), e[:].rearrange("p r i -> p (r i)"))
                    for r in range(4):
                        nc.tensor.matmul(
                            accs[r][0:HD + 1, :],
                            v_aug[:, jt, 4 * g + r, :],
                            e[:, r, :],
                            start=(jt == 0), stop=(jt == NJT - 1))
                # epilogue for this group: divide num rows by den row
                if debug and g == 0:
                    acc_dump = dpool.tile([128, 512], F32, tag="acc_dump")
                    nc.vector.tensor_copy(acc_dump[:], accs[0][:])
                    nc.gpsimd.dma_start(dbg["acc0"].ap(), acc_dump[:])
                rden4 = dpool.tile([4, 512], F32R, tag="rden4")
                den_sp = dpool.tile([128, 4, 4], F32, tag="den_sp",
                                    name=f"den_sp_{g}")
                for r in range(4):
                    den_row = dpool.tile([1, 512], F32, tag=f"den_row{r}",
                                         name=f"den_row_g{g}_{r}")
                    nc.vector.tensor_copy(den_row[:], accs[r][HD:HD + 1, :])
                    nc.gpsimd.dma_start(den_sp[:, r, :], den_row[:])
                rec_sp = dpool.tile([128, 4, 4], F32, tag="rec_sp",
                                    name=f"rec_sp_{g}")
                nc.vector.reciprocal(rec_sp[:], den_sp[:])
                for r in range(4):
                    nc.gpsimd.dma_start(rden4[r:r + 1, :], rec_sp[:, r, :])
                div_ps = ps_big.tile([128, 4, 512], F32, tag="s",
                                     name=f"div_ps_g{g}")
                nc.tensor.matmul(div_ps[:, 0, :], sel4_sb[:], rden4[:],
                                 start=True, stop=True)
                numsb = dpool.tile([128, 512], F32, tag="numsb")
                for r in range(4):
                    nc.vector.tensor_copy(numsb[32 * r:32 * (r + 1), :],
                                          accs[r][0:HD, :])
                at = dpool.tile([128, 512], F32R, tag=f"attT{g}")
                nc.vector.tensor_mul(at[:], numsb[:], div_ps[:, 0, :])
                attT.append(at)
                if debug and g == 0:
                    nc.gpsimd.dma_start(dbg["attT0"].ap(), at[:])

            # ---------------- pos rank-1 term: U[c', h] ----------------
            u_cols = dpool.tile([128, 2], F32R, tag="u_cols")
            for cc in range(2):
                pu = ps_av.tile([128, 512], F32, tag=f"av{cc}", name=f"pu_{cc}")
                for jt in range(NJT):
                    nc.tensor.matmul(
                        pu[:, 0:H],
                        v_nat[:, jt, cc * 128:(cc + 1) * 128],
                        w_sb[:, jt, :],
                        start=(jt == 0), stop=(jt == NJT - 1))
                for r in range(4):
                    h = 4 * cc + r
                    nc.vector.tensor_copy(u_cols[32 * r:32 * (r + 1), cc:cc + 1],
                                          pu[32 * r:32 * (r + 1), h:h + 1])

            if debug:
                nc.gpsimd.dma_start(dbg["ucols"].ap(), u_cols[:])
                nc.gpsimd.dma_start(
                    dbg["vaug0"].ap(),
                    v_aug[:, 0].rearrange("p h d -> p (h d)"))
            # const row = u'' @ Wop + bo   -> [1, 256]
            pc = ps_av.tile([128, 512], F32, tag="av2", name="pc")
            for cc in range(2):
                nc.tensor.matmul(pc[0:1, 0:C], u_cols[:, cc:cc + 1],
                                 wop_sb[:, cc, :], start=(cc == 0), stop=(cc == 1))
            const_row = dpool.tile([1, C], F32R, tag="const_row")
            nc.vector.tensor_add(const_row[:], pc[0:1, 0:C], bo_sb[:])
            if debug:
                nc.gpsimd.dma_start(dbg["crow"].ap(), const_row[:])

            # ---------------- final projection (natural layout) ----------------
            out_sb = dpool.tile([128, 4, C], F32, tag="out_sb")
            for ib in range(4):
                pf = ps_av.tile([128, 512], F32, tag=f"av{ib}")
                for g in range(2):
                    nc.tensor.matmul(
                        pf[:, 0:C],
                        attT[g][:, ib * 128:(ib + 1) * 128],
                        wop_sb[:, g, :],
                        start=(g == 0), stop=False)
                nc.tensor.matmul(
                    pf[:, 0:C],
                    ones_sb[0:1, :],
                    const_row[:],
                    start=False, stop=True)
                nc.vector.tensor_copy(out_sb[:, ib, :], pf[:, 0:C])

            nc.gpsimd.dma_start(
                out_d.ap().rearrange("(t p) c -> p t c", p=128), out_sb[:])

    nc.compile()
    return nc


def _host_prepare(x, pos, Ws, W1, b1, W2, b2, Wh, bh, gate, Wo, bo):
    """Host-side tiny pos-MLP + gate folding (float64)."""
    pos64 = pos.astype(np.float64)
    p = np.maximum(pos64 @ W1.astype(np.float64) + b1.astype(np.float64), 0.0)
    p = p @ W2.astype(np.float64) + b2.astype(np.float64)
    ph = p @ Wh.astype(np.float64)                      # [B, N, H]
    # pos softmax over keys j (i-independent): softmax_j(-ph[b, j, h])
    z = -ph
    z -= z.max(axis=1, keepdims=True)
    e = np.exp(z)
    wbar = e / e.sum(axis=1, keepdims=True)             # [B, N, H]
    g = 1.0 / (1.0 + np.exp(-gate.astype(np.float64)))  # [H]
    w_scaled = wbar * (g / (1.0 - g))[None, None, :]    # [B, N, H]
    row_scale = np.repeat(1.0 - g, HD)                  # [C]
    Wop = (Wo.astype(np.float64) * row_scale[:, None]).astype(np.float32)
    return w_scaled.astype(np.float32), Wop


def kernel(x, pos, Ws, W1, b1, W2, b2, Wh, bh, gate, Wo, bo):
    x = np.asarray(x, np.float32)
    pos = np.asarray(pos, np.float32)
    Ws = np.asarray(Ws, np.float32)
    W1 = np.asarray(W1, np.float32); b1 = np.asarray(b1, np.float32)
    W2 = np.asarray(W2, np.float32); b2 = np.asarray(b2, np.float32)
    Wh = np.asarray(Wh, np.float32); bh = np.asarray(bh, np.float32)
    gate = np.asarray(gate, np.float32)
    Wo = np.asarray(Wo, np.float32); bo = np.asarray(bo, np.float32)

    w_scaled, Wop = _host_prepare(x, pos, Ws, W1, b1, W2, b2, Wh, bh, gate, Wo, bo)

    profile = os.environ.get("KERNEL_PROFILE", "0") == "1"
    if profile:
        _install_profile_shim()

    debug = os.environ.get("KERNEL_DEBUG", "0") == "1"
    key = f"nc_dbg{int(debug)}"
    if key not in _PROGRAM_CACHE:
        _PROGRAM_CACHE[key] = _build_program(debug=debug)
    nc = _PROGRAM_CACHE[key]

    ones128 = np.ones((128, 128), np.float32)
    eye128 = np.eye(128, dtype=np.float32)
    sel4 = np.zeros((4, 128), np.float32)
    for r in range(4):
        sel4[r, 32 * r:32 * (r + 1)] = 1.0
    bo_2d = bo.reshape(1, C)

    in_maps = []
    for core in range(NCORES):
        b, iq = divmod(core, 4)
        shift = -IC * iq
        in_maps.append({
            "x": np.ascontiguousarray(np.roll(x[b], shift, axis=0)),
            "wbar": np.ascontiguousarray(np.roll(w_scaled[b], shift, axis=0)),
            "Ws": Ws, "Wop": Wop, "bo": bo_2d,
            "ones": ones128, "eye": eye128, "sel4": sel4,
        })

    res = run_bass_kernel_spmd(nc, in_maps, list(range(NCORES)), trace=profile)
    if profile:
        kernel.last_exec_time_ns = res.exec_time_ns
        kernel.last_mean_exec_time_ns = res.mean_exec_time_ns

    if debug:
        kernel.last_debug = res.results[0]

    out = np.empty((B, N, C), np.float32)
    for core in range(NCORES):
        b, iq = divmod(core, 4)
        out[b, IC * iq:IC * (iq + 1), :] = res.results[core]["out"]
    return out


# revision 21
# speedup vs baseline: 1.5141x; 1.2999x over previous
"""Trainium2 Bass kernel for nn_Attention_35605278883932.

Shape constants (hardcoded per the problem spec):
  B=2, N=2048, C=256, H=8, P=3, PH=32, hd=32.

Sharding: 8 cores = (batch b in {0,1}) x (query quarter iq in {0..3}).
Each core receives x[b] (and the pos-softmax weights) rolled so that its
512 query rows come first; attention is permutation-invariant over the
key axis, so rolling both queries and keys only changes fp summation
order. Each core computes the full attention for its 512 queries over
all 2048 keys and all 8 heads, plus the final projection; there is no
cross-core communication.

Math reductions applied (exact, not approximations):
  - pos_attn rows are i-independent: softmax_j(ph_i - ph_j + bh) =
    softmax_j(-ph_j), so pos_attn @ v is a rank-1 per-(b,h) vector u.
  - a = (1-g) attn + g pos_attn has row sums exactly 1, so the final
    renormalization is the identity.
  - per-head (1-g_h) is folded into rows of Wo; g_h/(1-g_h) into the
    host-computed pos weights; out = attn_out @ Wo' + const row.
The tiny pos MLP (P=3) and sigmoid(gate) run on host in float64; all
O(N*C) and O(N^2) work runs on the NeuronCores.
"""

import os
import numpy as np

import concourse.bacc as bacc
import concourse.mybir as mybir
import concourse.tile as tile
from concourse.bass_utils import run_bass_kernel_spmd

B, N, C, H, P = 2, 2048, 256, 8, 3
PH = C // 8
HD = C // H              # 32
NCORES = 8
IC = N // 4              # 512 queries per core
NJT = N // 128           # 16 key tiles
F32 = mybir.dt.float32
F32R = mybir.dt.float32r

_PROGRAM_CACHE = {}


def _install_profile_shim():
    """Register the NTFF profile hook missing from this image's antenv."""
    import sys, types
    try:
        from antenv.axon_hooks import get_axon_ntff_profile_hook  # noqa: F401
        return
    except ImportError:
        pass
    try:
        import trn_agent_boot.trn_boot as tb
        hook = tb._ntff_profile_via_ctypes("/opt/axon/libaxon_pjrt.so")
    except Exception:
        hook = None
    mod = types.ModuleType("antenv.axon_hooks")
    mod.get_axon_ntff_profile_hook = lambda: hook
    mod.set_axon_ntff_profile_hook = lambda h: None
    sys.modules["antenv.axon_hooks"] = mod
    from concourse import bass_utils
    bass_utils.upload_artifacts = lambda tmpdir: tmpdir


def _build_program(debug=False):
    """Emit the per-core Bass program (identical across cores)."""
    nc = bacc.Bacc("TRN2", target_bir_lowering=False, debug=False,
                   num_devices=NCORES)
    dbg = {}
    if debug:
        dbg["xT"] = nc.dram_tensor("dbg_xT", [128, 2 * NJT * 128], F32, kind="ExternalOutput")
        dbg["qkvT0"] = nc.dram_tensor("dbg_qkvT0", [128, N], F32, kind="ExternalOutput")
        dbg["E0"] = nc.dram_tensor("dbg_E0", [128, 4 * 512], F32, kind="ExternalOutput")
        dbg["acc0"] = nc.dram_tensor("dbg_acc0", [128, 512], F32, kind="ExternalOutput")
        dbg["attT0"] = nc.dram_tensor("dbg_attT0", [128, 512], F32, kind="ExternalOutput")
        dbg["ucols"] = nc.dram_tensor("dbg_ucols", [128, 2], F32, kind="ExternalOutput")
        dbg["crow"] = nc.dram_tensor("dbg_crow", [1, C], F32, kind="ExternalOutput")
        dbg["vaug0"] = nc.dram_tensor("dbg_vaug0", [128, H * (HD + 1)], F32, kind="ExternalOutput")

    x_d = nc.dram_tensor("x", [N, C], F32, kind="ExternalInput")
    w_d = nc.dram_tensor("wbar", [N, H], F32, kind="ExternalInput")
    ws_d = nc.dram_tensor("Ws", [C, C], F32, kind="ExternalInput")
    wop_d = nc.dram_tensor("Wop", [C, C], F32, kind="ExternalInput")
    bo_d = nc.dram_tensor("bo", [1, C], F32, kind="ExternalInput")
    ones_d = nc.dram_tensor("ones", [128, 128], F32, kind="ExternalInput")
    sel4_d = nc.dram_tensor("sel4", [4, 128], F32, kind="ExternalInput")
    eye_d = nc.dram_tensor("eye", [128, 128], F32, kind="ExternalInput")
    out_d = nc.dram_tensor("out", [IC, C], F32, kind="ExternalOutput")

    EXP_SCALE = 1.0 / np.sqrt(np.float32(HD))

    with tile.TileContext(nc) as tc:
        with (
            tc.tile_pool(name="consts", bufs=1) as cpool,
            tc.tile_pool(name="data", bufs=1) as dpool,
            tc.tile_pool(name="epool", bufs=4) as epool,
            tc.tile_pool(name="ps_big", bufs=1, space="PSUM") as ps_big,
            tc.tile_pool(name="ps_av", bufs=1, space="PSUM") as ps_av,
        ):
            # ---------------- constants ----------------
            ws_sb = cpool.tile([128, 2, C], F32R, tag="ws")       # [c-chunk, cc, c']
            nc.gpsimd.dma_start(ws_sb[:], ws_d.ap().rearrange("(cc p) c -> p cc c", p=128))
            wop_sb = cpool.tile([128, 2, C], F32R, tag="wop")
            nc.gpsimd.dma_start(wop_sb[:], wop_d.ap().rearrange("(cc p) c -> p cc c", p=128))
            bo_sb = cpool.tile([1, C], F32, tag="bo")
            nc.gpsimd.dma_start(bo_sb[:], bo_d.ap())
            ones_sb = cpool.tile([128, 128], F32R, tag="ones")
            nc.gpsimd.dma_start(ones_sb[:], ones_d.ap())
            eye_sb = cpool.tile([128, 128], F32R, tag="eye")
            nc.gpsimd.dma_start(eye_sb[:], eye_d.ap())
            sel4_sb = cpool.tile([4, 128], F32R, tag="sel4")
            nc.gpsimd.dma_start(sel4_sb[:], sel4_d.ap())
            w_sb = cpool.tile([128, NJT, H], F32R, tag="w")       # pos weights, j-major
            nc.gpsimd.dma_start(w_sb[:], w_d.ap().rearrange("(t p) h -> p t h", p=128))

            # ---------------- x load + transpose ----------------
            x_nat = dpool.tile([128, NJT, C], F32R, tag="x_nat")  # x[t*128+p, c]
            for q in range(4):
                nc.gpsimd.dma_start(
                    x_nat[:, 4 * q:4 * (q + 1), :],
                    x_d.ap().rearrange("(t p) c -> p t c", p=128)[:, 4 * q:4 * (q + 1), :])

            # xT[p, cc, t, q] = x[t*128+q, cc*128+p]
            xT = dpool.tile([128, 2, NJT, 128], F32R, tag="xT")
            for cc in range(2):
                for t0 in range(0, NJT, 2):
                    ptr = ps_big.tile([128, 2, 512], F32, tag=f"s{(t0 // 2) % 2}",
                                      name=f"ptr_{cc}_{t0}")
                    for k in range(2):
                        t = t0 + k
                        nc.tensor.transpose(
                            ptr[:, k, 0:128].bitcast(F32R),
                            x_nat[:, t, cc * 128:(cc + 1) * 128],
                            eye_sb[:])
                    nc.scalar.copy(
                        xT[:, cc, t0:t0 + 2, :],
                        ptr[:].rearrange("p k (f g) -> p k f g", f=4)[:, :, 0, :])

            # ---------------- qkvT (heads in partition quadrants) ----------------
            # qkvT_g[p=c' within half, i] ; head h = 4g + p//32
            qkvT = []
            for g in range(2):
                qt = dpool.tile([128, N], F32R, tag=f"qkvT{g}")
                for half in range(2):
                    pq = ps_big.tile([128, 2, 512], F32, tag=f"s{half}",
                                     name=f"pq_{g}_{half}")
                    for cc in range(2):
                        for s in (2 * half, 2 * half + 1):
                            nc.tensor.matmul(
                                pq[:, s - 2 * half, :],
                                ws_sb[:, cc, g * 128:(g + 1) * 128],
                                xT[:, cc].rearrange("p t q -> p (t q)")[:, s * 512:(s + 1) * 512],
                                start=(cc == 0), stop=(cc == 1))
                    eng = nc.vector.tensor_copy if half == 0 else nc.scalar.copy
                    eng(qt[:, half * 1024:(half + 1) * 1024],
                        pq[:].rearrange("p s i -> p (s i)"))
                qkvT.append(qt)
                if debug and g == 0:
                    nc.gpsimd.dma_start(dbg["qkvT0"].ap(), qt[:])
            if debug:
                nc.gpsimd.dma_start(
                    dbg["xT"].ap(),
                    xT[:].rearrange("p a t q -> p (a t q)"))

            # ---------------- qkv natural -> v_aug ----------------
            # v_aug[p=j within tile, t, h, 0:32] = v ; [..., 32] = 1.0
            v_aug = dpool.tile([128, NJT, H, HD + 1], F32R, tag="v_aug")
            v_nat = dpool.tile([128, NJT, C], F32R, tag="v_nat")
            nc.vector.tensor_copy(
                v_aug[:, :, :, HD:HD + 1].rearrange("p t h one -> p t (h one)"),
                ones_sb[:].rearrange("p (t h) -> p t h", t=NJT))
            for t in range(NJT):
                pv = ps_av.tile([128, 512], F32, tag=f"av{t % 4}", name=f"pv_{t}")
                for cc in range(2):
                    nc.tensor.matmul(
                        pv[:, 0:C],
                        xT[:, cc, t, :],
                        ws_sb[:, cc, :],
                        start=(cc == 0), stop=(cc == 1))
                nc.vector.tensor_copy(
                    v_aug[:, t, :, 0:HD],
                    pv[:, 0:C].rearrange("p (h d) -> p h d", h=H))
                nc.scalar.copy(v_nat[:, t, :], pv[:, 0:C])

            # ---------------- main attention: per head-group of 4 ----------------
            attT = []   # attn_out^T per group: [128 = (h_local, d), 512]
            for g in range(2):
                qt = qkvT[g]
                accs = [ps_av.tile([128, 512], F32, tag=f"av{r}", name=f"acc_g{g}_{r}")
                        for r in range(4)]
                for jt in range(NJT):
                    e = epool.tile([128, 4, 512], F32R, tag="E")
                    for half in range(2):
                        ps = ps_big.tile([128, 2, 512], F32, tag=f"s{half}",
                                         name=f"ps_g{g}_jt{jt}_h{half}")
                        for r in (2 * half, 2 * half + 1):
                            tp = (96, 0) if r == 3 else None
                            nc.tensor.matmul(
                                ps[:, r - 2 * half, :],
                                qt[32 * r:32 * (r + 1), jt * 128:(jt + 1) * 128],
                                qt[32 * r:32 * (r + 1), 0:IC],
                                start=True, stop=True, tile_position=tp)
                        nc.scalar.activation(
                            e[:, 2 * half:2 * half + 2, :].rearrange("p r i -> p (r i)"),
                            ps[:].rearrange("p r i -> p (r i)"),
                            mybir.ActivationFunctionType.Exp, scale=EXP_SCALE)
                    if debug and g == 0 and jt == 0:
                        nc.gpsimd.dma_start(dbg["E0"].ap(), e[:].rearrange("p r i -> p (r i)"))
                    for r in range(4):
                        nc.tensor.matmul(
                            accs[r][0:HD + 1, :],
                            v_aug[:, jt, 4 * g + r, :],
                            e[:, r, :],
                            start=(jt == 0), stop=(jt == NJT - 1))
                # epilogue for this group: divide num rows by den row
                if debug and g == 0:
                    acc_dump = dpool.tile([128, 512], F32, tag="acc_dump")
                    nc.vector.tensor_copy(acc_dump[:], accs[0][:])
                    nc.gpsimd.dma_start(dbg["acc0"].ap(), acc_dump[:])
                rden4 = dpool.tile([4, 512], F32R, tag="rden4")
                den_sp = dpool.tile([128, 4, 4], F32, tag="den_sp",
                                    name=f"den_sp_{g}")
                for r in range(4):
                    den_row = dpool.tile([1, 512], F32, tag=f"den_row{r}",
                                         name=f"den_row_g{g}_{r}")
                    nc.vector.tensor_copy(den_row[:], accs[r][HD:HD + 1, :])
                    nc.gpsimd.dma_start(den_sp[:, r, :], den_row[:])
                rec_sp = dpool.tile([128, 4, 4], F32, tag="rec_sp",
                                    name=f"rec_sp_{g}")
                nc.vector.reciprocal(rec_sp[:], den_sp[:])
                for r in range(4):
                    nc.gpsimd.dma_start(rden4[r:r + 1, :], rec_sp[:, r, :])
                div_ps = ps_big.tile([128, 2, 512], F32, tag="s0",
                                     name=f"div_ps_g{g}")
                nc.tensor.matmul(div_ps[:, 0, :], sel4_sb[:], rden4[:],
                                 start=True, stop=True)
                numsb = dpool.tile([128, 512], F32, tag="numsb")
                for r in range(4):
                    nc.vector.tensor_copy(numsb[32 * r:32 * (r + 1), :],
                                          accs[r][0:HD, :])
                at = dpool.tile([128, 512], F32R, tag=f"attT{g}")
                nc.vector.tensor_mul(at[:], numsb[:], div_ps[:, 0, :])
                attT.append(at)
                if debug and g == 0:
                    nc.gpsimd.dma_start(dbg["attT0"].ap(), at[:])

            # ---------------- pos rank-1 term: U[c', h] ----------------
            u_cols = dpool.tile([128, 2], F32R, tag="u_cols")
            for cc in range(2):
                pu = ps_av.tile([128, 512], F32, tag=f"av{cc}", name=f"pu_{cc}")
                for jt in range(NJT):
                    nc.tensor.matmul(
                        pu[:, 0:H],
                        v_nat[:, jt, cc * 128:(cc + 1) * 128],
                        w_sb[:, jt, :],
                        start=(jt == 0), stop=(jt == NJT - 1))
                for r in range(4):
                    h = 4 * cc + r
                    nc.vector.tensor_copy(u_cols[32 * r:32 * (r + 1), cc:cc + 1],
                                          pu[32 * r:32 * (r + 1), h:h + 1])

            if debug:
                nc.gpsimd.dma_start(dbg["ucols"].ap(), u_cols[:])
                nc.gpsimd.dma_start(
                    dbg["vaug0"].ap(),
                    v_aug[:, 0].rearrange("p h d -> p (h d)"))
            # const row = u'' @ Wop + bo   -> [1, 256]
            pc = ps_av.tile([128, 512], F32, tag="av2", name="pc")
            for cc in range(2):
                nc.tensor.matmul(pc[0:1, 0:C], u_cols[:, cc:cc + 1],
                                 wop_sb[:, cc, :], start=(cc == 0), stop=(cc == 1))
            const_row = dpool.tile([1, C], F32R, tag="const_row")
            nc.vector.tensor_add(const_row[:], pc[0:1, 0:C], bo_sb[:])
            if debug:
                nc.gpsimd.dma_start(dbg["crow"].ap(), const_row[:])

            # ---------------- final projection (natural layout) ----------------
            out_sb = dpool.tile([128, 4, C], F32, tag="out_sb")
            for ib in range(4):
                pf = ps_av.tile([128, 512], F32, tag=f"av{ib}")
                for g in range(2):
                    nc.tensor.matmul(
                        pf[:, 0:C],
                        attT[g][:, ib * 128:(ib + 1) * 128],
                        wop_sb[:, g, :],
                        start=(g == 0), stop=False)
                nc.tensor.matmul(
                    pf[:, 0:C],
                    ones_sb[0:1, :],
                    const_row[:],
                    start=False, stop=True)
                nc.vector.tensor_copy(out_sb[:, ib, :], pf[:, 0:C])

            nc.gpsimd.dma_start(
                out_d.ap().rearrange("(t p) c -> p t c", p=128), out_sb[:])

    nc.compile()
    return nc


def _host_prepare(x, pos, Ws, W1, b1, W2, b2, Wh, bh, gate, Wo, bo):
    """Host-side tiny pos-MLP + gate folding (float64)."""
    pos64 = pos.astype(np.float64)
    p = np.maximum(pos64 @ W1.astype(np.float64) + b1.astype(np.float64), 0.0)
    p = p @ W2.astype(np.float64) + b2.astype(np.float64)
    ph = p @ Wh.astype(np.float64)                      # [B, N, H]
    # pos softmax over keys j (i-independent): softmax_j(-ph[b, j, h])
    z = -ph
    z -= z.max(axis=1, keepdims=True)
    e = np.exp(z)
    wbar = e / e.sum(axis=1, keepdims=True)             # [B, N, H]
    g = 1.0 / (1.0 + np.exp(-gate.astype(np.float64)))  # [H]
    w_scaled = wbar * (g / (1.0 - g))[None, None, :]    # [B, N, H]
    row_scale = np.repeat(1.0 - g, HD)                  # [C]
    Wop = (Wo.astype(np.float64) * row_scale[:, None]).astype(np.float32)
    return w_scaled.astype(np.float32), Wop


def kernel(x, pos, Ws, W1, b1, W2, b2, Wh, bh, gate, Wo, bo):
    x = np.asarray(x, np.float32)
    pos = np.asarray(pos, np.float32)
    Ws = np.asarray(Ws, np.float32)
    W1 = np.asarray(W1, np.float32); b1 = np.asarray(b1, np.float32)
    W2 = np.asarray(W2, np.float32); b2 = np.asarray(b2, np.float32)
    Wh = np.asarray(Wh, np.float32); bh = np.asarray(bh, np.float32)
    gate = np.asarray(gate, np.float32)
    Wo = np.asarray(Wo, np.float32); bo = np.asarray(bo, np.float32)

    w_scaled, Wop = _host_prepare(x, pos, Ws, W1, b1, W2, b2, Wh, bh, gate, Wo, bo)

    profile = os.environ.get("KERNEL_PROFILE", "0") == "1"
    if profile:
        _install_profile_shim()

    debug = os.environ.get("KERNEL_DEBUG", "0") == "1"
    key = f"nc_dbg{int(debug)}"
    if key not in _PROGRAM_CACHE:
        _PROGRAM_CACHE[key] = _build_program(debug=debug)
    nc = _PROGRAM_CACHE[key]

    ones128 = np.ones((128, 128), np.float32)
    eye128 = np.eye(128, dtype=np.float32)
    sel4 = np.zeros((4, 128), np.float32)
    for r in range(4):
        sel4[r, 32 * r:32 * (r + 1)] = 1.0
    bo_2d = bo.reshape(1, C)

    in_maps = []
    for core in range(NCORES):
        b, iq = divmod(core, 4)
        shift = -IC * iq
        in_maps.append({
            "x": np.ascontiguousarray(np.roll(x[b], shift, axis=0)),
            "wbar": np.ascontiguousarray(np.roll(w_scaled[b], shift, axis=0)),
            "Ws": Ws, "Wop": Wop, "bo": bo_2d,
            "ones": ones128, "eye": eye128, "sel4": sel4,
        })

    res = run_bass_kernel_spmd(nc, in_maps, list(range(NCORES)), trace=profile)
    if profile:
        kernel.last_exec_time_ns = res.exec_time_ns
        kernel.last_mean_exec_time_ns = res.mean_exec_time_ns

    if debug:
        kernel.last_debug = res.results[0]

    out = np.empty((B, N, C), np.float32)
    for core in range(NCORES):
        b, iq = divmod(core, 4)
        out[b, IC * iq:IC * (iq + 1), :] = res.results[core]["out"]
    return out
